# revision 33
# baseline (speedup 1.0000x reference)
"""DSMIL bass kernel for 8 TRN2 NeuronCores.

Computation (reference DSMIL head, N=50000 instances, D=512, C=2 classes):
    inst_pred = x @ w_fc.T + b_fc
    top = argmax(inst_pred, axis=0);  q_max = x[top] @ w_q.T + b_q
    A = softmax((x @ w_q.T + b_q) @ q_max.T / sqrt(D), axis=0)
    B = A.T @ (x @ w_v.T + b_v);  C_out = einsum('cd,ocd->o', B, w_conv) + b_conv

Device strategy (instance dim sharded 8 ways, 6250 rows/core):
    - algebraic fusion: s = x @ W2 + u with W2 = G @ m_feat.T, G = w_q.T w_q / sqrt(D)
      and B = (A.T @ x) @ w_v.T + b_v (softmax columns sum to 1), so the
      O(N*D*D) GEMMs disappear; every pass over x costs N*D/128 PE cycles.
    - phase 1: SWDGE-batched x loads, bf16 cast, PE transposes to xT,
      inst_pred.T streamed into a [C, ns] tile; local argmax = max8 +
      max_index on DVE; candidate rows fetched by indirect DMA;
      AllGather of [val | feat row] (tiny).
    - phase 2: W2/u from G and the winner row (selected by a second
      max_index + indirect gather), s streamed in natural chunk form
      straight into a PSUM bank, exp on ACT (s range is safe: no max
      subtraction), Bx = P.T @ x, AllReduce of [z | Bx], normalize,
      B = . @ w_v.T + b_v, conv contraction.
"""

import math
import sys

if "/opt/trn_rl_repo" not in sys.path:
    sys.path.insert(0, "/opt/trn_rl_repo")

import numpy as np

N_CORES = 8
N_FULL = 50000
D = 512
C = 2
NS = N_FULL // N_CORES          # 6250 rows per core
SQRT_D = math.sqrt(float(D))

_compiled = {}


def _build(ns: int):
    import concourse.bass as bass
    import concourse.mybir as mybir
    import concourse.tile as tile
    from concourse import bacc
    from concourse.masks import make_identity

    f32 = mybir.dt.float32
    bf16 = mybir.dt.bfloat16
    i32 = mybir.dt.int32
    u32 = mybir.dt.uint32
    Alu = mybir.AluOpType
    Act = mybir.ActivationFunctionType
    Ax = mybir.AxisListType

    nt = (ns + 127) // 128           # 128-row chunks
    nb = (nt + 3) // 4               # 512-row blocks (of 4 chunks)
    last_rows = ns - (nt - 1) * 128  # valid rows in the last chunk

    nc = bacc.Bacc("TRN2", target_bir_lowering=False, debug=False,
                   num_devices=N_CORES, num_swdge_queues=4)

    x_h = nc.dram_tensor("x", [ns, D], f32, kind="ExternalInput")
    w_fc_h = nc.dram_tensor("w_fc", [C, D], f32, kind="ExternalInput")
    b_fc_h = nc.dram_tensor("b_fc", [C], f32, kind="ExternalInput")
    w_q_h = nc.dram_tensor("w_q", [D, D], f32, kind="ExternalInput")
    b_q_h = nc.dram_tensor("b_q", [D], f32, kind="ExternalInput")
    w_v_h = nc.dram_tensor("w_v", [D, D], f32, kind="ExternalInput")
    b_v_h = nc.dram_tensor("b_v", [D], f32, kind="ExternalInput")
    w_conv_h = nc.dram_tensor("w_conv", [C, C, D], f32, kind="ExternalInput")
    b_conv_h = nc.dram_tensor("b_conv", [C], f32, kind="ExternalInput")

    ip_h = nc.dram_tensor("ip", [C, ns], f32, kind="ExternalOutput")
    cout_h = nc.dram_tensor("cout", [1, C], f32, kind="ExternalOutput")
    bag_h = nc.dram_tensor("bag", [1, C * D], f32, kind="ExternalOutput")

    x_ap = x_h.ap()

    with tile.TileContext(nc) as tc:
        with (
            tc.tile_pool(name="pers", bufs=1) as pers,
            tc.tile_pool(name="stg", bufs=2) as stg,
            tc.tile_pool(name="ps1", bufs=1, space="PSUM") as ps1,
            tc.tile_pool(name="ps2", bufs=2, space="PSUM") as ps2,
            tc.tile_pool(name="dram", bufs=1, space="DRAM") as dpool,
        ):
            rg = [list(range(N_CORES))]

            # ---------------- constants -------------------------------
            ident = pers.tile([128, 128], f32)
            make_identity(nc, ident[:])
            ident_bf = pers.tile([128, 128], bf16)
            nc.vector.tensor_copy(ident_bf[:], ident[:])

            iota_p = pers.tile([128, 1], i32)    # value = partition index
            nc.gpsimd.iota(iota_p[:], pattern=[[0, 1]], base=0,
                           channel_multiplier=1)
            iota_pf = pers.tile([128, 1], f32)
            nc.vector.tensor_copy(iota_pf[:], iota_p[:])
            padneg = pers.tile([128, 1], f32)
            nc.vector.tensor_scalar(
                padneg[:], iota_pf[:], float(last_rows) - 0.5, -1.0e30,
                Alu.is_ge, Alu.mult)

            ones_col = pers.tile([128, 1], f32)
            nc.vector.memset(ones_col[:], 1.0)

            # ---------------- weights ---------------------------------
            w_fc_sb = pers.tile([C, D], f32)
            nc.sync.dma_start(w_fc_sb[:], w_fc_h.ap())
            b_fc_sb = pers.tile([C, 1], f32)
            nc.sync.dma_start(b_fc_sb[:], b_fc_h.ap().unsqueeze(1))
            b_conv_sb = pers.tile([C, 1], f32)
            nc.sync.dma_start(b_conv_sb[:], b_conv_h.ap().unsqueeze(1))
            b_v_sb = pers.tile([C, D], f32)
            nc.gpsimd.dma_start(
                b_v_sb[:],
                bass.AP(tensor=b_v_h, offset=0, ap=[[0, C], [1, D]]))

            # w_fcT (bf16) chunks er: [p, c] = w_fc[c, 128*er+p]
            w_fcT_bf = pers.tile([128, 4 * C], bf16)
            tpf = ps2.tile([128, 4 * C], f32, tag="scr", bufs=1)
            for er in range(4):
                nc.tensor.transpose(
                    tpf[:, er * C:(er + 1) * C],
                    w_fc_sb[:, er * 128:(er + 1) * 128], ident[0:C, 0:C])
            nc.vector.tensor_copy(w_fcT_bf[:], tpf[:])

            # w_conv transposed chunks: [p, o] = w_conv_flat[o, 128k+p]
            wcv_stage = stg.tile([C, C * D], f32, tag="stageA", bufs=1,
                                 name="wcv_stage")
            nc.sync.dma_start(
                wcv_stage[:], w_conv_h.ap().rearrange("o c d -> o (c d)"))
            w_convT = pers.tile([128, 8 * C], f32)
            tcv = ps2.tile([128, 8 * C], f32, tag="scr", bufs=1)
            for k in range(8):
                nc.tensor.transpose(
                    tcv[:, k * C:(k + 1) * C],
                    wcv_stage[:, k * 128:(k + 1) * 128], ident[0:C, 0:C])
            nc.vector.tensor_copy(w_convT[:], tcv[:])

            # w_vT chunks er ([128, 512]): [p, j] = w_v[j, 128*er+p]
            wv_full = pers.tile([128, 4 * D], f32)
            nc.sync.dma_start(
                wv_full[:], w_v_h.ap().rearrange("(r p) d -> p r d", p=128))
            w_vT = pers.tile([128, 4 * D], f32)
            for er in range(4):
                wvt_ps = ps1.tile([128, 512], f32, tag="W", name=f"wvt{er}")
                for dr in range(4):
                    nc.tensor.transpose(
                        wvt_ps[:, dr * 128:(dr + 1) * 128],
                        wv_full[:, dr * D + er * 128: dr * D + (er + 1) * 128],
                        ident[:])
                nc.vector.tensor_copy(
                    w_vT[:, er * D:(er + 1) * D], wvt_ps[:])

            w_vT_bf = pers.tile([128, 4 * D], bf16)
            nc.vector.tensor_copy(w_vT_bf[:], w_vT[:])

            # G = (w_q.T @ w_q) / sqrt(D) (bf16), h = w_q.T @ b_q / sqrt(D)
            b_q_col = pers.tile([128, 4], f32)
            nc.sync.dma_start(
                b_q_col[:], b_q_h.ap().rearrange("(r p) -> p r", p=128))
            b_q_bf = pers.tile([128, 4], bf16)
            nc.vector.tensor_copy(b_q_bf[:], b_q_col[:])
            wq_full = pers.tile([128, 4 * D], bf16)
            nc.gpsimd.dma_start(
                wq_full[:], w_q_h.ap().rearrange("(r p) d -> p r d", p=128))
            G_sb = pers.tile([128, 4 * D], bf16)
            h_sb = pers.tile([128, 4], bf16)
            h_ps = ps1.tile([128, 4], f32, tag="B0")
            for er in range(4):
                g_ps = ps1.tile([128, 512], f32, tag="W", name=f"g_ps{er}")
                for dr in range(4):
                    nc.tensor.matmul(
                        g_ps[:],
                        lhsT=wq_full[:, dr * D + er * 128:
                                     dr * D + (er + 1) * 128],
                        rhs=wq_full[:, dr * D:(dr + 1) * D],
                        start=(dr == 0), stop=(dr == 3))
                    nc.tensor.matmul(
                        h_ps[:, er:er + 1],
                        lhsT=wq_full[:, dr * D + er * 128:
                                     dr * D + (er + 1) * 128],
                        rhs=b_q_bf[:, dr:dr + 1],
                        start=(dr == 0), stop=(dr == 3))
                nc.scalar.mul(
                    G_sb[:, er * D:(er + 1) * D], g_ps[:], 1.0 / SQRT_D)
            nc.scalar.mul(h_sb[:], h_ps[:], 1.0 / SQRT_D)

            # ---------------- phase 1: x load + transpose + inst_pred --
            x_ext = pers.tile([128, nt * D], bf16)     # x natural (bf16)
            xT = [pers.tile([128, nt * 128], bf16, name=f"xT{d}",
                            tag=f"xT{d}") for d in range(4)]
            ipT_full = pers.tile([C, ns], f32)
            mblk = pers.tile([C, 8 * nb], f32)
            midx = pers.tile([C, 8 * nb], u32)

            for g in range(nb):
                t0, t1 = g * 4, min(g * 4 + 4, nt)
                xe = x_ext[:, t0 * D: t1 * D]
                r0, r1 = t0 * 128, min(t1 * 128, ns)
                if r1 - r0 == (t1 - t0) * 128:
                    nc.gpsimd.dma_start(
                        xe, x_ap[r0:r1, :].rearrange(
                            "(i p) d -> p i d", p=128))
                else:
                    pad0 = (last_rows // 32) * 32
                    nc.vector.memset(
                        x_ext[pad0:, (nt - 1) * D:], 0.0)
                    full_t = (r1 - r0) // 128
                    if full_t:
                        nc.gpsimd.dma_start(
                            x_ext[:, t0 * D: (t0 + full_t) * D],
                            x_ap[r0:r0 + full_t * 128, :].rearrange(
                                "(i p) d -> p i d", p=128))
                    rem = (r1 - r0) - full_t * 128
                    if rem:
                        nc.gpsimd.dma_start(
                            x_ext[:rem, (t0 + full_t) * D:],
                            x_ap[r0 + full_t * 128: r1, :])

                # PE transposes (bf16) -> xT for this group
                for half in range(2):
                    tp = ps1.tile([128, 1024], bf16, tag=f"X{half}",
                                  bufs=2 if half == 0 else 1,
                                  name=f"tp{half}")
                    for dd in range(2):
                        d = half * 2 + dd
                        for t in range(t0, t1):
                            nc.tensor.transpose(
                                tp[:, dd * 512 + (t - t0) * 128:
                                   dd * 512 + (t - t0 + 1) * 128],
                                x_ext[:, t * D + d * 128:
                                      t * D + (d + 1) * 128],
                                ident_bf[:])
                    for dd in range(2):
                        d = half * 2 + dd
                        eng = nc.vector if dd == 0 else nc.scalar
                        if dd == 0:
                            nc.vector.tensor_copy(
                                xT[d][:, t0 * 128: t1 * 128],
                                tp[:, dd * 512: dd * 512 + (t1 - t0) * 128])
                        else:
                            nc.scalar.copy(
                                xT[d][:, t0 * 128: t1 * 128],
                                tp[:, dd * 512: dd * 512 + (t1 - t0) * 128])

                # inst_pred.T for this block + per-block top8 and its index
                c0 = g * 512
                c1 = min(c0 + 512, nt * 128)
                w = c1 - c0
                rows_out = min(512, ns - c0)
                ip_ps = ps2.tile([C, 512], f32, tag="st")
                for er in range(4):
                    nc.tensor.matmul(
                        ip_ps[:, :w],
                        lhsT=w_fcT_bf[:, er * C:(er + 1) * C],
                        rhs=xT[er][:, c0:c1],
                        start=(er == 0), stop=(er == 3))
                nc.vector.tensor_scalar(
                    ipT_full[:, c0:c0 + rows_out], ip_ps[:, :rows_out],
                    b_fc_sb[:], None, Alu.add)
                nc.vector.max(
                    mblk[:, g * 8:(g + 1) * 8],
                    ipT_full[:, c0:c0 + rows_out])
                nc.vector.max_index(
                    midx[:, g * 8:(g + 1) * 8],
                    mblk[:, g * 8:(g + 1) * 8],
                    ipT_full[:, c0:c0 + rows_out])
            # inst_pred output: one contiguous DMA (host restores order)
            nc.scalar.dma_start(ip_h.ap(), ipT_full[:])

            # ---------------- local argmax + candidate gather ----------
            # global row = 512*b + midx[b] for the block b holding the max
            stage1 = pers.tile([C, 8 + D], f32)
            gmax8 = stage1[:, 0:8]
            nc.vector.max(gmax8, mblk[:])
            mb0 = mblk[:].rearrange("c (b k) -> c k b", k=8)[:, 0, :]
            mi0 = midx[:].rearrange("c (b k) -> c k b", k=8)[:, 0, :]
            midx_f = stg.tile([C, nb], f32, tag="midx_f", bufs=1)
            nc.vector.tensor_copy(midx_f[:], mi0)
            blockbase = pers.tile([C, nb], f32)
            bb_i = pers.tile([C, nb], i32)
            nc.gpsimd.iota(bb_i[:], pattern=[[512, nb]], base=0,
                           channel_multiplier=0)
            nc.vector.tensor_copy(blockbase[:], bb_i[:])
            gsel = stg.tile([C, nb], f32, tag="gsel", bufs=1)
            # (mblk0 == gmax) * (midx + 512*b), summed -> global argmax row
            nc.vector.tensor_add(gsel[:], midx_f[:], blockbase[:])
            oh = stg.tile([C, nb], f32, tag="oh", bufs=1)
            nc.vector.tensor_scalar(
                oh[:], mb0, stage1[:, 0:1], None, Alu.is_equal)
            nc.vector.tensor_mul(gsel[:], gsel[:], oh[:])
            gidx_f = stg.tile([C, 1], f32, tag="gidx_f", bufs=1)
            nc.vector.tensor_reduce(gidx_f[:], gsel[:], Ax.X, Alu.add)
            idx_i32 = pers.tile([C, 1], i32)
            nc.vector.tensor_copy(idx_i32[:], gidx_f[:])
            nc.gpsimd.indirect_dma_start(
                out=stage1[:, 8:], out_offset=None,
                in_=x_ap,
                in_offset=bass.IndirectOffsetOnAxis(ap=idx_i32[:, 0:1],
                                                    axis=0))

            # ---------------- AllGather 1: [val(top8) | feat row] ------
            ag1_in = dpool.tile([C, 8 + D], f32)
            ag1_out = dpool.tile([N_CORES * C, 8 + D], f32)
            nc.gpsimd.dma_start(ag1_in[:], stage1[:])
            nc.gpsimd.collective_compute(
                "AllGather", mybir.AluOpType.bypass, replica_groups=rg,
                ins=[ag1_in[:].opt()], outs=[ag1_out[:].opt()])

            # keep the PE's HAM clock warm across the collective window:
            # a dead-end accumulation chain gated on stage1 (ready at AG issue)
            warm_ps = ps2.tile([C, 512], f32, tag="st")
            for w_i in range(14):
                nc.tensor.matmul(
                    warm_ps[:], lhsT=stage1[:, 0:2], rhs=stage1[:, 8:],
                    start=(w_i == 0), stop=(w_i == 13))
            warm_sb = stg.tile([C, 512], f32, tag="warm_sb", bufs=1)
            nc.vector.tensor_copy(warm_sb[:], warm_ps[:])
            warm_dr = dpool.tile([C, 512], f32)
            nc.sync.dma_start(warm_dr[:], warm_sb[:])

            # winner rank by max_index over the 8 gathered vals, then
            # indirect-gather the winning [val | feat row].
            vals2 = stg.tile([C, N_CORES], f32, tag="vals2", bufs=1)
            nc.sync.dma_start(
                vals2[:],
                ag1_out[:].rearrange("(r c) f -> c r f", c=C)[:, :, 0])
            g8v = stg.tile([C, 8], f32, tag="g8v", bufs=1)
            nc.vector.max(g8v[:], vals2[:])
            widx = stg.tile([C, 8], u32, tag="widx", bufs=1)
            nc.vector.max_index(widx[:], g8v[:], vals2[:])
            wf = stg.tile([C, 1], f32, tag="wf", bufs=1)
            nc.vector.tensor_copy(wf[:], widx[:, 0:1])
            rowf = stg.tile([C, 1], f32, tag="rowf", bufs=1)
            # ag1_out row = 2*rank + class
            nc.vector.scalar_tensor_tensor(
                rowf[:], wf[:], 2.0, iota_pf[0:C, :], Alu.mult, Alu.add)
            rowi = stg.tile([C, 1], i32, tag="rowi", bufs=1)
            nc.vector.tensor_copy(rowi[:], rowf[:])
            m_feat_ext = pers.tile([C, 8 + D], f32)
            nc.gpsimd.indirect_dma_start(
                out=m_feat_ext[:], out_offset=None,
                in_=ag1_out[:],
                in_offset=bass.IndirectOffsetOnAxis(ap=rowi[:, 0:1], axis=0))

            # ---------------- W2 = G @ m_feat.T, u = h . m_feat --------
            m_featT = pers.tile([128, 4 * C], bf16)
            tmf = ps2.tile([128, 4 * C], f32, tag="scr", bufs=1)
            for er in range(4):
                nc.tensor.transpose(
                    tmf[:, er * C:(er + 1) * C],
                    m_feat_ext[:, 8 + er * 128: 8 + (er + 1) * 128],
                    ident[0:C, 0:C])
            nc.vector.tensor_copy(m_featT[:], tmf[:])

            W2_bf = pers.tile([128, 4 * C], bf16)
            w2_ps = ps2.tile([128, 4 * C], f32, tag="scr", bufs=1)
            for er in range(4):
                for dr in range(4):
                    nc.tensor.matmul(
                        w2_ps[:, er * C:(er + 1) * C],
                        lhsT=G_sb[:, dr * D + er * 128:
                                  dr * D + (er + 1) * 128],
                        rhs=m_featT[:, dr * C:(dr + 1) * C],
                        start=(dr == 0), stop=(dr == 3))
            nc.vector.tensor_copy(W2_bf[:], w2_ps[:])

            u_ps = ps2.tile([1, C], f32, tag="scr", bufs=1)
            for er in range(4):
                nc.tensor.matmul(
                    u_ps[:],
                    lhsT=h_sb[:, er:er + 1],
                    rhs=m_featT[:, er * C:(er + 1) * C],
                    start=(er == 0), stop=(er == 3))
            u_row = stg.tile([1, C], f32, tag="u_row", bufs=1)
            nc.vector.tensor_copy(u_row[:], u_ps[:])
            u_bc = pers.tile([128, C], f32)
            nc.gpsimd.partition_broadcast(u_bc[:], u_row[:])

            # ---------------- phase 2: s, exp, z, Bx -------------------
            Ppack = ps1.tile([128, 2 * nt], f32, tag="B0")
            for t in range(nt):
                for er in range(4):
                    nc.tensor.matmul(
                        Ppack[:, t * 2:(t + 1) * 2],
                        lhsT=xT[er][:, t * 128:(t + 1) * 128],
                        rhs=W2_bf[:, er * C:(er + 1) * C],
                        start=(er == 0), stop=(er == 3))
            pv = Ppack[:].rearrange("p (i c) -> p i c", c=2)
            nc.vector.tensor_tensor(
                pv, pv, u_bc[:].unsqueeze(1).to_broadcast([128, nt, C]),
                Alu.add)
            if last_rows < 128:
                nc.vector.tensor_add(
                    Ppack[:, (nt - 1) * 2:], Ppack[:, (nt - 1) * 2:],
                    padneg[:].to_broadcast([128, 2]))
            P_all = pers.tile([128, 2 * nt], bf16)
            nc.scalar.activation(P_all[:], Ppack[:], Act.Exp)

            # z and Bx
            P_r = P_all[:].rearrange("p (i c) -> p c i", c=2)
            zp2 = stg.tile([128, C], f32, tag="zp2", bufs=1)
            for c in range(C):
                nc.vector.tensor_reduce(
                    zp2[:, c:c + 1], P_r[:, c, :], Ax.X, Alu.add)
            z_ps = ps2.tile([C, 1], f32, tag="scr", bufs=1)
            nc.tensor.matmul(
                z_ps[:], lhsT=zp2[:], rhs=ones_col[:], start=True, stop=True)
            stage2 = pers.tile([C, 1 + D], f32)
            nc.vector.tensor_copy(stage2[:, 0:1], z_ps[:])

            bx_ps = ps1.tile([C, D], f32, tag="W")
            for t in range(nt):
                nc.tensor.matmul(
                    bx_ps[:],
                    lhsT=P_all[:, t * 2:(t + 1) * 2],
                    rhs=x_ext[:, t * D:(t + 1) * D],
                    start=(t == 0), stop=(t == nt - 1))
            nc.vector.tensor_copy(stage2[:, 1:], bx_ps[:])

            # ---------------- AllReduce 2: [z | Bx] --------------------
            ar2_in = dpool.tile([C, 1 + D], f32)
            ar2_out = dpool.tile([C, 1 + D], f32)
            nc.sync.dma_start(ar2_in[:], stage2[:])
            nc.gpsimd.collective_compute(
                "AllReduce", mybir.AluOpType.add, replica_groups=rg,
                ins=[ar2_in[:].opt()], outs=[ar2_out[:].opt()])
            sumt = pers.tile([C, 1 + D], f32)
            nc.sync.dma_start(sumt[:], ar2_out[:])

            # ---------------- B = (Bx @ w_v.T)/Z + b_v -----------------
            rz = stg.tile([C, 1], f32, tag="rz", bufs=1)
            nc.vector.reciprocal(rz[:], sumt[:, 0:1])

            bxnT = pers.tile([128, 4 * C], bf16)
            tbx = ps2.tile([128, 4 * C], f32, tag="scr", bufs=1)
            for er in range(4):
                nc.tensor.transpose(
                    tbx[:, er * C:(er + 1) * C],
                    sumt[:, 1 + er * 128: 1 + (er + 1) * 128],
                    ident[0:C, 0:C])
            nc.vector.tensor_copy(bxnT[:], tbx[:])

            bfin_ps = ps2.tile([C, D], f32, tag="scr", bufs=1)
            for er in range(4):
                nc.tensor.matmul(
                    bfin_ps[:],
                    lhsT=bxnT[:, er * C:(er + 1) * C],
                    rhs=w_vT_bf[:, er * D:(er + 1) * D],
                    start=(er == 0), stop=(er == 3))
            bfin = pers.tile([C, D], f32)
            nc.vector.scalar_tensor_tensor(
                bfin[:], bfin_ps[:], rz[:], b_v_sb[:], Alu.mult, Alu.add)
            nc.scalar.dma_start(
                bag_h.ap().rearrange("o (c d) -> (o c) d", c=C), bfin[:])

            # C_out via PE: lhsT = w_convT chunks, rhs = bfinT columns
            bfinT = pers.tile([128, 4 * C], f32)
            tbf = ps2.tile([128, 4 * C], f32, tag="scr", bufs=1)
            for er in range(4):
                nc.tensor.transpose(
                    tbf[:, er * C:(er + 1) * C],
                    bfin[:, er * 128:(er + 1) * 128], ident[0:C, 0:C])
            nc.vector.tensor_copy(bfinT[:], tbf[:])
            cout_ps = ps2.tile([C, 1], f32, tag="scr", bufs=1)
            for k in range(8):
                c, er = k // 4, k % 4
                nc.tensor.matmul(
                    cout_ps[:],
                    lhsT=w_convT[:, k * C:(k + 1) * C],
                    rhs=bfinT[:, er * C + c: er * C + c + 1],
                    start=(k == 0), stop=(k == 7))
            cout_sb = pers.tile([C, 1], f32)
            nc.vector.tensor_add(cout_sb[:], cout_ps[:], b_conv_sb[:])
            nc.scalar.dma_start(
                cout_h.ap().rearrange("o c -> (o c) ()"), cout_sb[:])

    nc.compile()
    return nc


def kernel(x, w_fc, b_fc, w_q, b_q, w_v, b_v, w_conv, b_conv):
    from concourse import bass_utils

    x = np.ascontiguousarray(np.asarray(x, dtype=np.float32))
    feat = x.reshape(N_FULL, D)
    ns = NS

    if "nc" not in _compiled:
        _compiled["nc"] = _build(ns)
    nc = _compiled["nc"]

    common = {
        "w_fc": np.ascontiguousarray(np.asarray(w_fc, np.float32)),
        "b_fc": np.ascontiguousarray(np.asarray(b_fc, np.float32)),
        "w_q": np.ascontiguousarray(np.asarray(w_q, np.float32)),
        "b_q": np.ascontiguousarray(np.asarray(b_q, np.float32)),
        "w_v": np.ascontiguousarray(np.asarray(w_v, np.float32)),
        "b_v": np.ascontiguousarray(np.asarray(b_v, np.float32)),
        "w_conv": np.ascontiguousarray(np.asarray(w_conv, np.float32)),
        "b_conv": np.ascontiguousarray(np.asarray(b_conv, np.float32)),
    }
    in_maps = []
    for c in range(N_CORES):
        m = dict(common)
        m["x"] = np.ascontiguousarray(feat[c * ns:(c + 1) * ns])
        in_maps.append(m)

    res = bass_utils.run_bass_kernel_spmd(
        nc, in_maps, core_ids=list(range(N_CORES)))
    outs = res.results

    inst_pred = np.concatenate(
        [np.ascontiguousarray(outs[c]["ip"].T) for c in range(N_CORES)],
        axis=0)
    c_out = outs[0]["cout"]
    bag = outs[0]["bag"]
    return (c_out, bag, inst_pred)


# revision 34
# speedup vs baseline: 1.1380x; 1.1380x over previous
"""DSMIL bass kernel for 8 TRN2 NeuronCores.

Computation (reference DSMIL head, N=50000 instances, D=512, C=2 classes):
    inst_pred = x @ w_fc.T + b_fc
    top = argmax(inst_pred, axis=0);  q_max = x[top] @ w_q.T + b_q
    A = softmax((x @ w_q.T + b_q) @ q_max.T / sqrt(D), axis=0)
    B = A.T @ (x @ w_v.T + b_v);  C_out = einsum('cd,ocd->o', B, w_conv) + b_conv

Device strategy (instance dim sharded 8 ways, 6250 rows/core):
    - algebraic fusion: s = x @ W2 + u with W2 = G @ m_feat.T, G = w_q.T w_q / sqrt(D)
      and B = (A.T @ x) @ w_v.T + b_v (softmax columns sum to 1), so the
      O(N*D*D) GEMMs disappear; every pass over x costs N*D/128 PE cycles.
    - phase 1: SWDGE-batched x loads, bf16 cast, PE transposes to xT,
      inst_pred.T streamed into a [C, ns] tile; local argmax = max8 +
      max_index on DVE; candidate rows fetched by indirect DMA;
      AllGather of [val | feat row] (tiny).
    - phase 2: W2/u from G and the winner row (selected by a second
      max_index + indirect gather), s streamed in natural chunk form
      straight into a PSUM bank, exp on ACT (s range is safe: no max
      subtraction), Bx = P.T @ x, AllReduce of [z | Bx], normalize,
      B = . @ w_v.T + b_v, conv contraction.
"""

import math
import sys

if "/opt/trn_rl_repo" not in sys.path:
    sys.path.insert(0, "/opt/trn_rl_repo")

import numpy as np

N_CORES = 8
N_FULL = 50000
D = 512
C = 2
NS = N_FULL // N_CORES          # 6250 rows per core
SQRT_D = math.sqrt(float(D))

_compiled = {}


def _build(ns: int):
    import concourse.bass as bass
    import concourse.mybir as mybir
    import concourse.tile as tile
    from concourse import bacc
    from concourse.masks import make_identity

    f32 = mybir.dt.float32
    bf16 = mybir.dt.bfloat16
    i32 = mybir.dt.int32
    u32 = mybir.dt.uint32
    Alu = mybir.AluOpType
    Act = mybir.ActivationFunctionType
    Ax = mybir.AxisListType

    nt = (ns + 127) // 128           # 128-row chunks
    nb = (nt + 3) // 4               # 512-row blocks (of 4 chunks)
    last_rows = ns - (nt - 1) * 128  # valid rows in the last chunk

    nc = bacc.Bacc("TRN2", target_bir_lowering=False, debug=False,
                   num_devices=N_CORES, num_swdge_queues=4)

    x_h = nc.dram_tensor("x", [ns, D], f32, kind="ExternalInput")
    w_fc_h = nc.dram_tensor("w_fc", [C, D], f32, kind="ExternalInput")
    b_fc_h = nc.dram_tensor("b_fc", [C], f32, kind="ExternalInput")
    w_q_h = nc.dram_tensor("w_q", [D, D], f32, kind="ExternalInput")
    b_q_h = nc.dram_tensor("b_q", [D], f32, kind="ExternalInput")
    w_v_h = nc.dram_tensor("w_v", [D, D], f32, kind="ExternalInput")
    b_v_h = nc.dram_tensor("b_v", [D], f32, kind="ExternalInput")
    w_conv_h = nc.dram_tensor("w_conv", [C, C, D], f32, kind="ExternalInput")
    b_conv_h = nc.dram_tensor("b_conv", [C], f32, kind="ExternalInput")

    ip_h = nc.dram_tensor("ip", [C, ns], f32, kind="ExternalOutput")
    cout_h = nc.dram_tensor("cout", [1, C], f32, kind="ExternalOutput")
    bag_h = nc.dram_tensor("bag", [1, C * D], f32, kind="ExternalOutput")

    x_ap = x_h.ap()

    with tile.TileContext(nc) as tc:
        with (
            tc.tile_pool(name="pers", bufs=1) as pers,
            tc.tile_pool(name="stg", bufs=2) as stg,
            tc.tile_pool(name="ps1", bufs=1, space="PSUM") as ps1,
            tc.tile_pool(name="ps2", bufs=2, space="PSUM") as ps2,
            tc.tile_pool(name="dram", bufs=1, space="DRAM") as dpool,
        ):
            rg = [list(range(N_CORES))]

            # ---------------- constants -------------------------------
            ident = pers.tile([128, 128], f32)
            make_identity(nc, ident[:])
            ident_bf = pers.tile([128, 128], bf16)
            nc.vector.tensor_copy(ident_bf[:], ident[:])

            iota_p = pers.tile([128, 1], i32)    # value = partition index
            nc.gpsimd.iota(iota_p[:], pattern=[[0, 1]], base=0,
                           channel_multiplier=1)
            iota_pf = pers.tile([128, 1], f32)
            nc.vector.tensor_copy(iota_pf[:], iota_p[:])
            padneg = pers.tile([128, 1], f32)
            nc.vector.tensor_scalar(
                padneg[:], iota_pf[:], float(last_rows) - 0.5, -1.0e30,
                Alu.is_ge, Alu.mult)

            ones_col = pers.tile([128, 1], f32)
            nc.vector.memset(ones_col[:], 1.0)

            # ---------------- weights ---------------------------------
            w_fc_sb = pers.tile([C, D], f32)
            nc.sync.dma_start(w_fc_sb[:], w_fc_h.ap())
            b_fc_sb = pers.tile([C, 1], f32)
            nc.sync.dma_start(b_fc_sb[:], b_fc_h.ap().unsqueeze(1))
            b_conv_sb = pers.tile([C, 1], f32)
            nc.sync.dma_start(b_conv_sb[:], b_conv_h.ap().unsqueeze(1))
            b_v_sb = pers.tile([C, D], f32)
            nc.gpsimd.dma_start(
                b_v_sb[:],
                bass.AP(tensor=b_v_h, offset=0, ap=[[0, C], [1, D]]))

            # w_fcT (bf16) chunks er: [p, c] = w_fc[c, 128*er+p]
            w_fcT_bf = pers.tile([128, 4 * C], bf16)
            tpf = ps2.tile([128, 4 * C], f32, tag="scr", bufs=1)
            for er in range(4):
                nc.tensor.transpose(
                    tpf[:, er * C:(er + 1) * C],
                    w_fc_sb[:, er * 128:(er + 1) * 128], ident[0:C, 0:C])
            nc.vector.tensor_copy(w_fcT_bf[:], tpf[:])

            # w_conv transposed chunks: [p, o] = w_conv_flat[o, 128k+p]
            wcv_stage = stg.tile([C, C * D], f32, tag="stageA", bufs=1,
                                 name="wcv_stage")
            nc.sync.dma_start(
                wcv_stage[:], w_conv_h.ap().rearrange("o c d -> o (c d)"))
            w_convT = pers.tile([128, 8 * C], f32)
            tcv = ps2.tile([128, 8 * C], f32, tag="scr", bufs=1)
            for k in range(8):
                nc.tensor.transpose(
                    tcv[:, k * C:(k + 1) * C],
                    wcv_stage[:, k * 128:(k + 1) * 128], ident[0:C, 0:C])
            nc.vector.tensor_copy(w_convT[:], tcv[:])

            # w_vT chunks er ([128, 512]): [p, j] = w_v[j, 128*er+p]
            wv_full = pers.tile([128, 4 * D], f32)
            nc.sync.dma_start(
                wv_full[:], w_v_h.ap().rearrange("(r p) d -> p r d", p=128))
            w_vT = pers.tile([128, 4 * D], f32)
            for er in range(4):
                wvt_ps = ps1.tile([128, 512], f32, tag="W", name=f"wvt{er}")
                for dr in range(4):
                    nc.tensor.transpose(
                        wvt_ps[:, dr * 128:(dr + 1) * 128],
                        wv_full[:, dr * D + er * 128: dr * D + (er + 1) * 128],
                        ident[:])
                nc.vector.tensor_copy(
                    w_vT[:, er * D:(er + 1) * D], wvt_ps[:])

            w_vT_bf = pers.tile([128, 4 * D], bf16)
            nc.vector.tensor_copy(w_vT_bf[:], w_vT[:])

            # G = (w_q.T @ w_q) / sqrt(D) (bf16), h = w_q.T @ b_q / sqrt(D)
            b_q_col = pers.tile([128, 4], f32)
            nc.sync.dma_start(
                b_q_col[:], b_q_h.ap().rearrange("(r p) -> p r", p=128))
            b_q_bf = pers.tile([128, 4], bf16)
            nc.vector.tensor_copy(b_q_bf[:], b_q_col[:])
            wq_full = pers.tile([128, 4 * D], bf16)
            nc.gpsimd.dma_start(
                wq_full[:], w_q_h.ap().rearrange("(r p) d -> p r d", p=128))
            G_sb = pers.tile([128, 4 * D], bf16)
            h_sb = pers.tile([128, 4], bf16)
            h_ps = ps1.tile([128, 4], f32, tag="B0")
            for er in range(4):
                g_ps = ps1.tile([128, 512], f32, tag="W", name=f"g_ps{er}")
                for dr in range(4):
                    nc.tensor.matmul(
                        g_ps[:],
                        lhsT=wq_full[:, dr * D + er * 128:
                                     dr * D + (er + 1) * 128],
                        rhs=wq_full[:, dr * D:(dr + 1) * D],
                        start=(dr == 0), stop=(dr == 3))
                    nc.tensor.matmul(
                        h_ps[:, er:er + 1],
                        lhsT=wq_full[:, dr * D + er * 128:
                                     dr * D + (er + 1) * 128],
                        rhs=b_q_bf[:, dr:dr + 1],
                        start=(dr == 0), stop=(dr == 3))
                nc.scalar.mul(
                    G_sb[:, er * D:(er + 1) * D], g_ps[:], 1.0 / SQRT_D)
            nc.scalar.mul(h_sb[:], h_ps[:], 1.0 / SQRT_D)

            # ---------------- phase 1: x load + transpose + inst_pred --
            x_ext = pers.tile([128, nt * D], bf16)     # x natural (bf16)
            xT = [pers.tile([128, nt * 128], bf16, name=f"xT{d}",
                            tag=f"xT{d}") for d in range(4)]
            ipT_full = pers.tile([C, ns], f32)
            mblk = pers.tile([C, 8 * nb], f32)
            midx = pers.tile([C, 8 * nb], u32)

            for g in range(nb):
                t0, t1 = g * 4, min(g * 4 + 4, nt)
                xe = x_ext[:, t0 * D: t1 * D]
                r0, r1 = t0 * 128, min(t1 * 128, ns)
                if r1 - r0 == (t1 - t0) * 128:
                    nc.gpsimd.dma_start(
                        xe, x_ap[r0:r1, :].rearrange(
                            "(i p) d -> p i d", p=128))
                else:
                    pad0 = (last_rows // 32) * 32
                    nc.vector.memset(
                        x_ext[pad0:, (nt - 1) * D:], 0.0)
                    full_t = (r1 - r0) // 128
                    if full_t:
                        nc.gpsimd.dma_start(
                            x_ext[:, t0 * D: (t0 + full_t) * D],
                            x_ap[r0:r0 + full_t * 128, :].rearrange(
                                "(i p) d -> p i d", p=128))
                    rem = (r1 - r0) - full_t * 128
                    if rem:
                        nc.gpsimd.dma_start(
                            x_ext[:rem, (t0 + full_t) * D:],
                            x_ap[r0 + full_t * 128: r1, :])

                # PE transposes (bf16) -> xT for this group
                for half in range(2):
                    tp = ps1.tile([128, 1024], bf16, tag=f"X{half}",
                                  bufs=2 if half == 0 else 1,
                                  name=f"tp{half}")
                    for dd in range(2):
                        d = half * 2 + dd
                        for t in range(t0, t1):
                            nc.tensor.transpose(
                                tp[:, dd * 512 + (t - t0) * 128:
                                   dd * 512 + (t - t0 + 1) * 128],
                                x_ext[:, t * D + d * 128:
                                      t * D + (d + 1) * 128],
                                ident_bf[:])
                    for dd in range(2):
                        d = half * 2 + dd
                        eng = nc.vector if dd == 0 else nc.scalar
                        if dd == 0:
                            nc.vector.tensor_copy(
                                xT[d][:, t0 * 128: t1 * 128],
                                tp[:, dd * 512: dd * 512 + (t1 - t0) * 128])
                        else:
                            nc.scalar.copy(
                                xT[d][:, t0 * 128: t1 * 128],
                                tp[:, dd * 512: dd * 512 + (t1 - t0) * 128])

                # inst_pred.T for this block + per-block top8 and its index
                c0 = g * 512
                c1 = min(c0 + 512, nt * 128)
                w = c1 - c0
                rows_out = min(512, ns - c0)
                ip_ps = ps2.tile([C, 512], f32, tag="st")
                for er in range(4):
                    nc.tensor.matmul(
                        ip_ps[:, :w],
                        lhsT=w_fcT_bf[:, er * C:(er + 1) * C],
                        rhs=xT[er][:, c0:c1],
                        start=(er == 0), stop=(er == 3))
                nc.vector.tensor_scalar(
                    ipT_full[:, c0:c0 + rows_out], ip_ps[:, :rows_out],
                    b_fc_sb[:], None, Alu.add)
                nc.vector.max(
                    mblk[:, g * 8:(g + 1) * 8],
                    ipT_full[:, c0:c0 + rows_out])
                nc.vector.max_index(
                    midx[:, g * 8:(g + 1) * 8],
                    mblk[:, g * 8:(g + 1) * 8],
                    ipT_full[:, c0:c0 + rows_out])
            # inst_pred output: one contiguous DMA (host restores order)
            nc.scalar.dma_start(ip_h.ap(), ipT_full[:])

            # ---------------- local argmax + candidate gather ----------
            # global row = 512*b + midx[b] for the block b holding the max
            stage1 = pers.tile([C, 8 + D], f32)
            gmax8 = stage1[:, 0:8]
            nc.vector.max(gmax8, mblk[:])
            mb0 = mblk[:].rearrange("c (b k) -> c k b", k=8)[:, 0, :]
            mi0 = midx[:].rearrange("c (b k) -> c k b", k=8)[:, 0, :]
            midx_f = stg.tile([C, nb], f32, tag="midx_f", bufs=1)
            nc.vector.tensor_copy(midx_f[:], mi0)
            blockbase = pers.tile([C, nb], f32)
            bb_i = pers.tile([C, nb], i32)
            nc.gpsimd.iota(bb_i[:], pattern=[[512, nb]], base=0,
                           channel_multiplier=0)
            nc.vector.tensor_copy(blockbase[:], bb_i[:])
            gsel = stg.tile([C, nb], f32, tag="gsel", bufs=1)
            # (mblk0 == gmax) * (midx + 512*b), summed -> global argmax row
            nc.vector.tensor_add(gsel[:], midx_f[:], blockbase[:])
            oh = stg.tile([C, nb], f32, tag="oh", bufs=1)
            nc.vector.tensor_scalar(
                oh[:], mb0, stage1[:, 0:1], None, Alu.is_equal)
            nc.vector.tensor_mul(gsel[:], gsel[:], oh[:])
            gidx_f = stg.tile([C, 1], f32, tag="gidx_f", bufs=1)
            nc.vector.tensor_reduce(gidx_f[:], gsel[:], Ax.X, Alu.add)
            idx_i32 = pers.tile([C, 1], i32)
            nc.vector.tensor_copy(idx_i32[:], gidx_f[:])
            nc.gpsimd.indirect_dma_start(
                out=stage1[:, 8:], out_offset=None,
                in_=x_ap,
                in_offset=bass.IndirectOffsetOnAxis(ap=idx_i32[:, 0:1],
                                                    axis=0))

            # ---------------- AllGather 1: [val(top8) | feat row] ------
            ag1_in = dpool.tile([C, 8 + D], f32)
            ag1_out = dpool.tile([N_CORES * C, 8 + D], f32)
            nc.gpsimd.dma_start(ag1_in[:], stage1[:])
            nc.gpsimd.collective_compute(
                "AllGather", mybir.AluOpType.bypass, replica_groups=rg,
                ins=[ag1_in[:].opt()], outs=[ag1_out[:].opt()])

            # winner rank by max_index over the 8 gathered vals, then
            # indirect-gather the winning [val | feat row].
            vals2 = stg.tile([C, N_CORES], f32, tag="vals2", bufs=1)
            nc.sync.dma_start(
                vals2[:],
                ag1_out[:].rearrange("(r c) f -> c r f", c=C)[:, :, 0])
            g8v = stg.tile([C, 8], f32, tag="g8v", bufs=1)
            nc.vector.max(g8v[:], vals2[:])
            widx = stg.tile([C, 8], u32, tag="widx", bufs=1)
            nc.vector.max_index(widx[:], g8v[:], vals2[:])
            wf = stg.tile([C, 1], f32, tag="wf", bufs=1)
            nc.vector.tensor_copy(wf[:], widx[:, 0:1])
            rowf = stg.tile([C, 1], f32, tag="rowf", bufs=1)
            # ag1_out row = 2*rank + class
            nc.vector.scalar_tensor_tensor(
                rowf[:], wf[:], 2.0, iota_pf[0:C, :], Alu.mult, Alu.add)
            rowi = stg.tile([C, 1], i32, tag="rowi", bufs=1)
            nc.vector.tensor_copy(rowi[:], rowf[:])
            m_feat_ext = pers.tile([C, 8 + D], f32)
            nc.gpsimd.indirect_dma_start(
                out=m_feat_ext[:], out_offset=None,
                in_=ag1_out[:],
                in_offset=bass.IndirectOffsetOnAxis(ap=rowi[:, 0:1], axis=0))

            # ---------------- W2 = G @ m_feat.T, u = h . m_feat --------
            m_featT = pers.tile([128, 4 * C], bf16)
            tmf = ps2.tile([128, 4 * C], f32, tag="scr", bufs=1)
            for er in range(4):
                nc.tensor.transpose(
                    tmf[:, er * C:(er + 1) * C],
                    m_feat_ext[:, 8 + er * 128: 8 + (er + 1) * 128],
                    ident[0:C, 0:C])
            nc.vector.tensor_copy(m_featT[:], tmf[:])

            W2_bf = pers.tile([128, 4 * C], bf16)
            w2_ps = ps2.tile([128, 4 * C], f32, tag="scr", bufs=1)
            for er in range(4):
                for dr in range(4):
                    nc.tensor.matmul(
                        w2_ps[:, er * C:(er + 1) * C],
                        lhsT=G_sb[:, dr * D + er * 128:
                                  dr * D + (er + 1) * 128],
                        rhs=m_featT[:, dr * C:(dr + 1) * C],
                        start=(dr == 0), stop=(dr == 3))
            nc.vector.tensor_copy(W2_bf[:], w2_ps[:])

            u_ps = ps2.tile([1, C], f32, tag="scr", bufs=1)
            for er in range(4):
                nc.tensor.matmul(
                    u_ps[:],
                    lhsT=h_sb[:, er:er + 1],
                    rhs=m_featT[:, er * C:(er + 1) * C],
                    start=(er == 0), stop=(er == 3))
            u_row = stg.tile([1, C], f32, tag="u_row", bufs=1)
            nc.vector.tensor_copy(u_row[:], u_ps[:])
            u_bc = pers.tile([128, C], f32)
            nc.gpsimd.partition_broadcast(u_bc[:], u_row[:])

            # ---------------- phase 2: s, exp, z, Bx -------------------
            Ppack = ps1.tile([128, 2 * nt], f32, tag="B0")
            for t in range(nt):
                for er in range(4):
                    nc.tensor.matmul(
                        Ppack[:, t * 2:(t + 1) * 2],
                        lhsT=xT[er][:, t * 128:(t + 1) * 128],
                        rhs=W2_bf[:, er * C:(er + 1) * C],
                        start=(er == 0), stop=(er == 3))
            pv = Ppack[:].rearrange("p (i c) -> p i c", c=2)
            nc.vector.tensor_tensor(
                pv, pv, u_bc[:].unsqueeze(1).to_broadcast([128, nt, C]),
                Alu.add)
            if last_rows < 128:
                nc.vector.tensor_add(
                    Ppack[:, (nt - 1) * 2:], Ppack[:, (nt - 1) * 2:],
                    padneg[:].to_broadcast([128, 2]))
            P_all = pers.tile([128, 2 * nt], bf16)
            nc.scalar.activation(P_all[:], Ppack[:], Act.Exp)

            # z and Bx
            P_r = P_all[:].rearrange("p (i c) -> p c i", c=2)
            zp2 = stg.tile([128, C], f32, tag="zp2", bufs=1)
            for c in range(C):
                nc.vector.tensor_reduce(
                    zp2[:, c:c + 1], P_r[:, c, :], Ax.X, Alu.add)
            z_ps = ps2.tile([C, 1], f32, tag="scr", bufs=1)
            nc.tensor.matmul(
                z_ps[:], lhsT=zp2[:], rhs=ones_col[:], start=True, stop=True)
            stage2 = pers.tile([C, 1 + D], f32)
            nc.vector.tensor_copy(stage2[:, 0:1], z_ps[:])

            bx_ps = ps1.tile([C, D], f32, tag="W")
            for t in range(nt):
                nc.tensor.matmul(
                    bx_ps[:],
                    lhsT=P_all[:, t * 2:(t + 1) * 2],
                    rhs=x_ext[:, t * D:(t + 1) * D],
                    start=(t == 0), stop=(t == nt - 1))
            nc.vector.tensor_copy(stage2[:, 1:], bx_ps[:])

            # ---------------- AllReduce 2: [z | Bx] --------------------
            ar2_in = dpool.tile([C, 1 + D], f32)
            ar2_out = dpool.tile([C, 1 + D], f32)
            nc.sync.dma_start(ar2_in[:], stage2[:])
            nc.gpsimd.collective_compute(
                "AllReduce", mybir.AluOpType.add, replica_groups=rg,
                ins=[ar2_in[:].opt()], outs=[ar2_out[:].opt()])
            sumt = pers.tile([C, 1 + D], f32)
            nc.sync.dma_start(sumt[:], ar2_out[:])

            # ---------------- B = (Bx @ w_v.T)/Z + b_v -----------------
            rz = stg.tile([C, 1], f32, tag="rz", bufs=1)
            nc.vector.reciprocal(rz[:], sumt[:, 0:1])

            bxnT = pers.tile([128, 4 * C], bf16)
            tbx = ps2.tile([128, 4 * C], f32, tag="scr", bufs=1)
            for er in range(4):
                nc.tensor.transpose(
                    tbx[:, er * C:(er + 1) * C],
                    sumt[:, 1 + er * 128: 1 + (er + 1) * 128],
                    ident[0:C, 0:C])
            nc.vector.tensor_copy(bxnT[:], tbx[:])

            bfin_ps = ps2.tile([C, D], f32, tag="scr", bufs=1)
            for er in range(4):
                nc.tensor.matmul(
                    bfin_ps[:],
                    lhsT=bxnT[:, er * C:(er + 1) * C],
                    rhs=w_vT_bf[:, er * D:(er + 1) * D],
                    start=(er == 0), stop=(er == 3))
            bfin = pers.tile([C, D], f32)
            nc.vector.scalar_tensor_tensor(
                bfin[:], bfin_ps[:], rz[:], b_v_sb[:], Alu.mult, Alu.add)
            nc.scalar.dma_start(
                bag_h.ap().rearrange("o (c d) -> (o c) d", c=C), bfin[:])

            # C_out via PE: lhsT = w_convT chunks, rhs = bfinT columns
            bfinT = pers.tile([128, 4 * C], f32)
            tbf = ps2.tile([128, 4 * C], f32, tag="scr", bufs=1)
            for er in range(4):
                nc.tensor.transpose(
                    tbf[:, er * C:(er + 1) * C],
                    bfin[:, er * 128:(er + 1) * 128], ident[0:C, 0:C])
            nc.vector.tensor_copy(bfinT[:], tbf[:])
            cout_ps = ps2.tile([C, 1], f32, tag="scr", bufs=1)
            for k in range(8):
                c, er = k // 4, k % 4
                nc.tensor.matmul(
                    cout_ps[:],
                    lhsT=w_convT[:, k * C:(k + 1) * C],
                    rhs=bfinT[:, er * C + c: er * C + c + 1],
                    start=(k == 0), stop=(k == 7))
            cout_sb = pers.tile([C, 1], f32)
            nc.vector.tensor_add(cout_sb[:], cout_ps[:], b_conv_sb[:])
            nc.scalar.dma_start(
                cout_h.ap().rearrange("o c -> (o c) ()"), cout_sb[:])

    nc.compile()
    return nc


def kernel(x, w_fc, b_fc, w_q, b_q, w_v, b_v, w_conv, b_conv):
    from concourse import bass_utils

    x = np.ascontiguousarray(np.asarray(x, dtype=np.float32))
    feat = x.reshape(N_FULL, D)
    ns = NS

    if "nc" not in _compiled:
        _compiled["nc"] = _build(ns)
    nc = _compiled["nc"]

    common = {
        "w_fc": np.ascontiguousarray(np.asarray(w_fc, np.float32)),
        "b_fc": np.ascontiguousarray(np.asarray(b_fc, np.float32)),
        "w_q": np.ascontiguousarray(np.asarray(w_q, np.float32)),
        "b_q": np.ascontiguousarray(np.asarray(b_q, np.float32)),
        "w_v": np.ascontiguousarray(np.asarray(w_v, np.float32)),
        "b_v": np.ascontiguousarray(np.asarray(b_v, np.float32)),
        "w_conv": np.ascontiguousarray(np.asarray(w_conv, np.float32)),
        "b_conv": np.ascontiguousarray(np.asarray(b_conv, np.float32)),
    }
    in_maps = []
    for c in range(N_CORES):
        m = dict(common)
        m["x"] = np.ascontiguousarray(feat[c * ns:(c + 1) * ns])
        in_maps.append(m)

    res = bass_utils.run_bass_kernel_spmd(
        nc, in_maps, core_ids=list(range(N_CORES)))
    outs = res.results

    inst_pred = np.concatenate(
        [np.ascontiguousarray(outs[c]["ip"].T) for c in range(N_CORES)],
        axis=0)
    c_out = outs[0]["cout"]
    bag = outs[0]["bag"]
    return (c_out, bag, inst_pred)
